# revision 54
# baseline (speedup 1.0000x reference)
"""Trainium2 Bass kernel for nn_ActionDetokenizer (gnn_message_passing).

Computes: out[b, j, k] = sum_d x[b, j+1, d] * W[j, d, k] + bias[j, k]
  x: [65536, 13, 256] f32, W: [12, 256, 2] f32, b: [12, 2] f32 -> out [65536, 12, 2] f32

Strategy (pure data parallel over batch, 8 cores):
  - Host: shard batch across 8 cores; cast x to fp16 (halves the HBM
    stream; ~3e-4 scale-relative output error vs the 2e-2 gate) and relayout
    the needed slice to d-major [12, C*128, 8192] so the contraction dim (d)
    lands on SBUF partitions (the TensorEngine contracts along partitions).
    The tiny weight stack (also fp16) is replicated to every core.
  - Device: stream x HBM->SBUF (memory-bound: ~50.3 MB/core) as one 4 MiB
    DMA per joint, alternating the two HWDGE rings (SP/ACT).  The DGE
    spreads a DMA's packets over the 16 hardware DMA engines by ROW index
    within the transfer (~26.7 GB/s per engine, ~427 GB/s aggregate); the
    device HAM throttle duty-cycles engines to half rate in k=4/n=8
    windows that lengthen as the run heats up, so the achieved stream rate
    decays from 427 toward ~330 late in the kernel.
  - For each (joint, 512-batch chunk) accumulate the two 128-contraction
    matmuls into PSUM with W[j] chunks stationary; 4 consecutive chunks
    are column-tiled via tile_position to PE column groups {0,32,64,96}
    so their matmuls run concurrently.  ONE wide drain per PSUM bank
    (partitions 0..97; these engines are free-dim-bound so [98,512] costs
    the same as [2,512]) fuses the per-(j,k) bias via a stripe-replicated
    bias AP into an f16 ring accumulator, alternating ACT/DVE.
  - Out path: stripe rows live at partitions {32g+k}, so any direct out
    DMA spans only 2 rows -> 2 DMA engines (row-indexed engine mapping),
    ~27 GB/s max, and out traffic on the x rings head-of-line blocks
    later x triggers behind unsatisfied drain waits.  Instead a tiny
    selection matmul (lhsT = 0/1 matrix, PSUM banks memset once so dead
    rows are exactly 0.0) gathers the 8 stripe rows onto contiguous PSUM
    partitions 0..8, compact drains copy them (bit-exact) into an f16
    ring, and ONE per-tile out DMA [8 rows, 2048] rides the rings with a
    2-tile lag: 8 DMA engines, ~160ns/packet, fully hidden in the stream.
  - Everything stays on HWDGE (only SP/ACT can trigger it; SWDGE/gpsimd
    DMAs are intermittently corrupt on this HW).  Output leaves as
    [8 rows = (g,k), J*bl/4 cols = (j,grp,b)] f16, host upcasts+permutes.

  Measured on trn2 (8 cores, full problem): ~146 us HW exec (best of
  several runs; HAM throttle adds +/-5 us run-to-run) vs ~124 us
  no-throttle DMA bound (51 MB/core at 427 GB/s + ~6 us NEFF preamble).
  Scale-relative absmax ~5e-4 (f16 in, f16 out).  History: 181.5 us
  staged baseline -> 164.6 reproduced -> 154.3 (mid-stream 2-row outs)
  -> 145.7 (compacted 8-row outs, this design).

Compute paths (KERNEL_PATH env or _PATH):
  f16  : fp16 x and W matmuls (default; ~5e-4 scale-rel err)
  f32  : exact fp32 matmuls, legacy out path (~284 us)
  f32r : float32r matmuls (no column tiling: ISA check rejects the combo)
  hilo : x and W split into bf16 hi+lo on host; 3-term bf16 matmuls
         (xhi@whi + xhi@wlo + xlo@whi), ~5e-6 scale-rel err
"""

import os

import numpy as np

M_CORES = 8
B_FULL = 65536
BL = B_FULL // M_CORES  # 8192 batch rows per core
J = 12  # joints
D = 256  # embed dim
K = 2  # outputs per joint
P = 128  # SBUF partitions / d-chunk
C = D // P  # 2 d-chunks
NB_TILE = 4096  # batch columns per SBUF x tile
OT_TILE = 2048  # batch columns per output staging tile
N_MM = 512  # batch columns per matmul (fp32 moving-operand max / PSUM bank)
G = 4  # column-tiling stripes (concurrent matmuls at PE col groups 32*g)

_PATH = os.environ.get("KERNEL_PATH", "f16")

_CACHE = {}


def _layout(path, bl):
    import os
    """Tile geometry shared by _build / _prep / _gather.

    nb (batch columns per x DMA) targets ~4 MiB per transfer:
      f32: planes=2 x 128 x 4096 x 4B = 4MiB;  f16: planes=2 x 128 x 8192 x 2B;
      hilo: planes=4 x 128 x 4096 x 2B.
    """
    default_nb = 8192 if path == "f16" else NB_TILE
    nb = min(int(os.environ.get("KNB", default_nb)), bl)
    assert bl % nb == 0 and nb % N_MM == 0
    n_bh = bl // nb
    n_n = nb // N_MM
    # float32r + tile_position fails walrus codegen; no striping there
    g_n = 1 if path == "f32r" else min(G, n_n)
    n_grp = n_n // g_n
    return nb, n_bh, n_n, g_n, n_grp


def _build(path, bl):
    import os
    import concourse.bacc as bacc
    import concourse.mybir as mybir
    from concourse.tile import TileContext

    f32 = mybir.dt.float32
    xdt = {
        "f32": f32,
        "f32r": mybir.dt.float32r,
        "hilo": mybir.dt.bfloat16,
        "f16": mybir.dt.float16,
    }[path]
    # number of (term) planes stacked along the x free dim per joint:
    # f32/f32r: C d-chunks; hilo: 2 sources (hi, lo) x C d-chunks
    n_src = 2 if path == "hilo" else 1
    planes = n_src * C

    # Bacc (not plain Bass): its compile() legalizes multi-wait instructions
    # into event semaphores / ldweights waits, which walrus codegen requires
    # (at most one wait command per compute instruction on TRN2).
    nc = bacc.Bacc("TRN2", target_bir_lowering=False, dynamic_dma_scratch_size=8192)

    # x relayout per core: [J, planes*P, bl]
    x_dram = nc.dram_tensor("xt", [J, planes * P, bl], xdt, kind="ExternalInput")
    # weights: [P, J*n_wsrc*C*K]; hilo has whi,wlo stacked along free dim
    n_wsrc = 2 if path == "hilo" else 1
    w_dram = nc.dram_tensor(
        "wt", [P, J * n_wsrc * C * K], xdt, kind="ExternalInput"
    )
    # bias replicated to the stripe partitions: row 32*g + k holds b[j, k]
    bias_dram = nc.dram_tensor("biasr", [P, J], f32, kind="ExternalInput")

    nb, n_bh, n_n, g_n, n_grp_ = _layout(path, bl)

    # New f16 scheme: one persistent SBUF out accumulator (no opool
    # recycling, whose buffer reuse made late drains wait on the
    # end-deferred out DMAs and serialized the pipeline tail) + f16
    # drains (halves out bytes).  The DGE assigns a DMA's packets to
    # hardware DMA engines by ROW index within the transfer, so a
    # 2-partition out DMA pins all its packets on engines 0-1 (~27 GB/s
    # each) no matter how it's split.  Fix: a tiny selection matmul
    # (lhsT = 0/1 matrix) gathers the K*g_n stripe rows {32g+k} onto
    # contiguous partitions, compact-drained to spare rows of the same
    # accumulator; the single trailing [K*g_n, J*bl/g_n] out DMA is then
    # fully linear and spreads over K*g_n DMA engines.
    new_scheme = path == "f16"
    odt = xdt if new_scheme else f32
    n_cp = K * G  # compacted out rows

    if new_scheme:
        out_dram = nc.dram_tensor(
            "out", [n_cp, J * n_bh * n_grp_ * N_MM], odt, kind="ExternalOutput"
        )
        sel_dram = nc.dram_tensor("sel", [P, n_cp], xdt, kind="ExternalInput")
    else:
        out_dram = nc.dram_tensor(
            "out", [J, g_n, K, bl // g_n], odt, kind="ExternalOutput"
        )

    # matmul term sequence per (j, n): (w_src, x_src, c)
    if path == "hilo":
        # hi@whi, lo@whi, hi@wlo  (xlo@wlo dropped: ~2^-18 relative)
        terms = [(0, 0), (0, 1), (1, 0)]
    else:
        terms = [(0, 0)]
    mms = [(ws, xs, c) for (ws, xs) in terms for c in range(C)]

    def w_col(j, ws, c):
        # column offset of W chunk (j, ws, c) in w_dram/w_sb
        return ((j * n_wsrc + ws) * C + c) * K

    def x_plane(xs, c):
        return xs * C + c

    # partition rows covered by one wide drain: stripes live at 32*g..32*g+K
    hi_p = 32 * (g_n - 1) + K

    with TileContext(nc) as tc:
        if new_scheme:
            # 4 x bufs (128K) + ring ot_full (24K) + ring ot_c (24K) fits
            # the ~208K usable per partition
            x_bufs = int(os.environ.get("KXB", "4"))
        else:
            x_bufs = int(os.environ.get("KXB", "6" if nb <= 4096 else "4"))
        ring = 6  # ot ring slots (per-tile); out lag 2 << ring
        ps_bufs = 6 if new_scheme else 8
        with (
            tc.tile_pool(name="wpool", bufs=1) as wpool,
            tc.tile_pool(name="xpool", bufs=x_bufs) as xpool,
            tc.tile_pool(name="opool", bufs=5) as opool,
            tc.tile_pool(name="pspool", bufs=ps_bufs, space="PSUM") as pspool,
            tc.tile_pool(name="pscpool", bufs=2, space="PSUM") as pscpool,
        ):
            w_sb = wpool.tile([P, J * n_wsrc * C * K], xdt, tag="w")
            bias_sb = wpool.tile([P, J], f32, tag="bias")
            ot_full = None
            sel_sb = None
            if new_scheme:
                # ring accumulators, one slot per in-flight tile:
                # ot_full holds wide-drain output (stripe rows 32g+k),
                # ot_c the compacted rows 0..n_cp (engine partition starts
                # must be 32-aligned, so compact rows can't share
                # ot_full's partition space -- its aligned starts all
                # collide with stripe rows).  WAR recycling trails by
                # `ring` tiles, far behind the out-DMA lag of 2.
                ot_full = wpool.tile(
                    [P, ring * n_grp_ * N_MM], odt, tag="ot"
                )
                ot_c = wpool.tile(
                    [P, ring * n_grp_ * N_MM], odt, tag="otc"
                )
                sel_sb = wpool.tile([P, n_cp], xdt, tag="sel")
            n_grp = n_grp_  # n-chunk groups per (j, bh)
            tiles = [(j, bh) for j in range(J) for bh in range(n_bh)]
            kwb = os.environ.get("KWB", "scalar")
            pre_xt = {}
            if new_scheme:
                # hoist the first two x-tile triggers ABOVE the w/bias/sel
                # loads: both rings start streaming at t0 and the 18 KiB of
                # weights queue behind x1's 4 MiB on the ACT ring, landing
                # long before tile 0's data (which gates the first matmul
                # anyway).  Program order keeps w/bias/sel writes ahead of
                # every compute read.
                for pidx in range(min(2, len(tiles))):
                    pj, pbh = tiles[pidx]
                    xt = xpool.tile([P, planes * nb], xdt, tag="x")
                    xt3 = xt.rearrange("p (pl b) -> p pl b", pl=planes)
                    src3 = x_dram[
                        pj, :, pbh * nb : (pbh + 1) * nb
                    ].rearrange("(pl p) b -> p pl b", p=P)
                    peng = nc.sync if pidx % 2 == 0 else nc.scalar
                    peng.dma_start(out=xt3[:, :, :], in_=src3[:, :, :])
                    pre_xt[pidx] = xt
            if kwb == "scalar":
                # weight/bias loads lead the ACT ring (x tile 0 leads SP)
                nc.scalar.dma_start(out=w_sb[:, :], in_=w_dram[:, :])
                nc.scalar.dma_start(out=bias_sb[:, :], in_=bias_dram[:, :])
                if new_scheme:
                    nc.scalar.dma_start(out=sel_sb[:, :], in_=sel_dram[:, :])
            if new_scheme:
                # one-time PSUM sanitize: the wide drains copy ALL of rows
                # 0..hi_p out of each bank, and the selection matmul then
                # multiplies those rows by 0 -- stale non-finite PSUM
                # content would turn 0*Inf into NaN.  Memset the stripe
                # banks once; matmuls only ever write stripe rows after
                # this, so dead rows stay exactly 0.0 forever.  Runs on
                # DVE under the first x-tile DMA window.
                for _ in range(ps_bufs):
                    t = pspool.tile([P, N_MM], f32, tag="ps")
                    nc.vector.memset(t[0:hi_p, :], 0.0)

            # Engine instruction streams execute in order, so an out-DMA
            # trigger still waiting on its drain would also hold up every
            # later x-load trigger queued behind it on that engine.  Defer
            # each tile's out DMAs by OUT_LAG tiles in program order: by then
            # its drain has long completed and the trigger never waits.
            # (gpsimd/SWDGE out DMAs would avoid this entirely but are
            # intermittently corrupt on HW; keep everything on HWDGE.)
            out_lag = int(os.environ.get("KLAG", "0"))
            new_lag = int(os.environ.get("KLAG2", "2"))
            pending = {}

            def _emit_out_c(idx, cur):
                # out DMA for tile idx ([n_cp rows, n_grp*N_MM] from its
                # ot_c ring slot -> n_cp DMA engines, ~160ns/packet),
                # queued right after x trigger `cur` on the same ring; its
                # compact drains finished ~a tile ago, so the trigger
                # never stalls the ring.
                if idx not in pending:
                    return
                j, bh = pending.pop(idx)
                dcol0 = (j * n_bh + bh) * n_grp * N_MM
                scol0 = (idx % ring) * n_grp * N_MM
                # always ACT: SYNC stays a pure x ring so no out trigger
                # can ever sit ahead of an x load there
                nc.scalar.dma_start(
                    out=out_dram[:, dcol0 : dcol0 + n_grp * N_MM],
                    in_=ot_c[0:n_cp, scol0 : scol0 + n_grp * N_MM],
                )

            def _emit_out(idx):
                if idx not in pending:
                    return
                j, bh, ot = pending.pop(idx)
                # same-parity ring as x tile idx+out_lag, queued right after
                # that x trigger
                o_eng = nc.scalar if idx % 2 == 0 else nc.sync
                for g in range(g_n):
                    o_eng.dma_start(
                        out=out_dram[
                            j,
                            g,
                            :,
                            bh * n_grp * N_MM : (bh + 1) * n_grp * N_MM,
                        ],
                        in_=ot[32 * g : 32 * g + K, :],
                    )

            for idx, (j, bh) in enumerate(tiles):
                # one DMA brings all planes (d-chunks x hi/lo) for this
                # (j, bh): [P, planes*nb]
                prefetched = idx in pre_xt
                if prefetched:
                    xt = pre_xt.pop(idx)
                else:
                    xt = xpool.tile([P, planes * nb], xdt, tag="x")
                xt3 = xt.rearrange("p (pl b) -> p pl b", pl=planes)
                src = x_dram[j, :, bh * nb : (bh + 1) * nb]
                src3 = src.rearrange("(pl p) b -> p pl b", p=P)
                eng = nc.sync if idx % 2 == 0 else nc.scalar
                last = idx == len(tiles) - 1
                if prefetched:
                    pass  # trigger already emitted in the prefetch block
                elif new_scheme and last and n_grp > 1:
                    # split the final tile's load per n-chunk group so its
                    # matmuls start after the first slice lands instead of
                    # after the whole 4 MiB.  ALL subs ride SYNC: the ACT
                    # ring carries the tail tiles' compact drains, whose
                    # waits would head-of-line block these triggers.
                    gb = nb // n_grp
                    for grp in range(n_grp):
                        nc.sync.dma_start(
                            out=xt3[:, :, grp * gb : (grp + 1) * gb],
                            in_=src3[:, :, grp * gb : (grp + 1) * gb],
                        )
                else:
                    eng.dma_start(out=xt3[:, :, :], in_=src3[:, :, :])
                if idx == 0 and kwb == "sync":
                    # w/bias ride SP behind x0 so ACT starts x1 immediately
                    nc.sync.dma_start(out=w_sb[:, :], in_=w_dram[:, :])
                    nc.sync.dma_start(out=bias_sb[:, :], in_=bias_dram[:, :])
                if new_scheme:
                    _emit_out_c(idx - new_lag, idx)
                else:
                    _emit_out(idx - out_lag)
                # stripe layout: PSUM/SBUF partition rows 32*g hold the
                # output of n-chunk n = grp*G + g; the G stripes' matmuls
                # run CONCURRENTLY in disjoint PE column groups.
                if new_scheme:
                    ot = ot_full
                    ocol0 = (idx % ring) * n_grp * N_MM
                else:
                    ot = opool.tile([P, n_grp * N_MM], f32, tag="o")
                    ocol0 = 0
                for grp in range(n_grp):
                    ps = pspool.tile([P, N_MM], f32, tag="ps")
                    for i, (ws, xs, c) in enumerate(mms):
                        pl = x_plane(xs, c)
                        wc = w_col(j, ws, c)
                        for g in range(g_n):
                            n = grp * g_n + g
                            col = pl * nb + n * N_MM
                            nc.tensor.matmul(
                                ps[32 * g : 32 * g + K, :],
                                lhsT=w_sb[:, wc : wc + K],
                                rhs=xt[:, col : col + N_MM],
                                start=(i == 0),
                                stop=(i == len(mms) - 1),
                                tile_position=(0, 32 * g),
                            )
                    # ONE wide PSUM->SBUF drain per group covering all
                    # stripes (partitions 0..hi_p; the rows between the
                    # stripes are dead weight but these engines are
                    # free-dim-bound, so a [98,512] op costs the same as
                    # [2,512]).  Per-(j,k) bias is fused via the
                    # stripe-replicated bias AP.  Alternate ACT / DVE.
                    psl = ps[0:hi_p, :]
                    osl = ot[
                        0:hi_p,
                        ocol0 + grp * N_MM : ocol0 + (grp + 1) * N_MM,
                    ]
                    if (bh * n_grp + grp) % 2 == 0:
                        nc.scalar.activation(
                            out=osl,
                            in_=psl,
                            func=mybir.ActivationFunctionType.Identity,
                            bias=bias_sb[0:hi_p, j : j + 1],
                            scale=1.0,
                        )
                    else:
                        nc.vector.tensor_scalar_add(
                            out=osl,
                            in0=psl,
                            scalar1=bias_sb[0:hi_p, j : j + 1],
                        )
                    if new_scheme and last:
                        # interleaved compaction for the FINAL tile: group
                        # g's select+compact chain completes as soon as its
                        # own sub-slice has landed, so after the last byte
                        # arrives only group n_grp-1's chain remains
                        ocol = ocol0 + grp * N_MM
                        psc = pscpool.tile([P, N_MM], f32, tag="psc")
                        nc.tensor.matmul(
                            psc[0:n_cp, :],
                            lhsT=sel_sb[0:hi_p, 0:n_cp],
                            rhs=ot_full[0:hi_p, ocol : ocol + N_MM],
                            start=True,
                            stop=True,
                        )
                        nc.scalar.activation(
                            out=ot_c[0:n_cp, ocol : ocol + N_MM],
                            in_=psc[0:n_cp, :],
                            func=mybir.ActivationFunctionType.Identity,
                            scale=1.0,
                        )
                if new_scheme:
                    # compaction pass (after ALL this tile's stripe groups,
                    # so the selection matmuls never stall the PE stream on
                    # a drain that hasn't happened yet): gather the K*g_n
                    # stripe rows of each group chunk onto contiguous PSUM
                    # partitions 0..n_cp, then drain them (exact f16 copy;
                    # sel rows are 0/1 and dead rows are 0.0) into the
                    # compact rows of the ring.  Mid-stream tiles split the
                    # drains ACT/DVE (opposite the wide drains' parity) to
                    # balance engines; the LAST 3 tiles put all compacts on
                    # ACT and let ACT itself trigger their out DMA inline:
                    # engine streams are in-order, so those triggers carry
                    # no event-wait chains (which cost ~3.4us per trigger
                    # when the ring-emitted outs fire at the bare tail).
                    tail_tile = idx >= len(tiles) - 3
                    for grp in range(n_grp) if not last else ():
                        ocol = ocol0 + grp * N_MM
                        psc = pscpool.tile([P, N_MM], f32, tag="psc")
                        nc.tensor.matmul(
                            psc[0:n_cp, :],
                            lhsT=sel_sb[0:hi_p, 0:n_cp],
                            rhs=ot_full[0:hi_p, ocol : ocol + N_MM],
                            start=True,
                            stop=True,
                        )
                        cdst = ot_c[0:n_cp, ocol : ocol + N_MM]
                        if tail_tile or (bh * n_grp + grp) % 2 == 1:
                            nc.scalar.activation(
                                out=cdst,
                                in_=psc[0:n_cp, :],
                                func=mybir.ActivationFunctionType.Identity,
                                scale=1.0,
                            )
                        else:
                            nc.vector.tensor_copy(
                                out=cdst, in_=psc[0:n_cp, :]
                            )
                    if tail_tile:
                        dcol0 = (j * n_bh + bh) * n_grp * N_MM
                        nc.scalar.dma_start(
                            out=out_dram[:, dcol0 : dcol0 + n_grp * N_MM],
                            in_=ot_c[0:n_cp, ocol0 : ocol0 + n_grp * N_MM],
                        )
                    else:
                        pending[idx] = (j, bh)
                else:
                    pending[idx] = (j, bh, ot)
            if new_scheme:
                for i, idx in enumerate(sorted(pending)):
                    _emit_out_c(idx, i)
            else:
                for idx in sorted(pending):
                    _emit_out(idx)
    nc.compile()
    return nc


def _get_nc(path, bl):
    key = (path, bl)
    if key not in _CACHE:
        _CACHE[key] = _build(path, bl)
    return _CACHE[key]


def _split_hilo(a):
    import ml_dtypes

    hi = a.astype(ml_dtypes.bfloat16)
    lo = (a - hi.astype(np.float32)).astype(ml_dtypes.bfloat16)
    return hi, lo


def _prep_core_inputs(x, W, b, path, n_cores, bl):
    """Shard batch across cores; relayout x slice to [J, planes*P, bl]."""
    # W chunks: [P, J*n_wsrc*C*K], wt[d, ((j*n_wsrc+ws)*C+c)*K + k]
    wt32 = W.reshape(J, C, P, K).transpose(2, 0, 1, 3)  # [P, J, C, K]
    if path == "hilo":
        hi, lo = _split_hilo(np.ascontiguousarray(wt32))  # [P, J, C, K] each
        wt = np.stack([hi, lo], axis=2)  # [P, J, 2, C, K]
        wt = np.ascontiguousarray(wt.reshape(P, J * 2 * C * K))
    elif path == "f16":
        wt = np.ascontiguousarray(wt32.reshape(P, J * C * K)).astype(np.float16)
    else:
        wt = np.ascontiguousarray(wt32.reshape(P, J * C * K))
    # bias replicated to stripe partitions: row 32*g + k = b[j, k]
    bias = np.zeros((P, J), dtype=np.float32)
    for g in range(P // 32):
        bias[32 * g : 32 * g + K, :] = b.T


    xsrc = x
    if path == "f16":
        # cast once up front: halves the bytes the per-core transposes move
        xsrc = x[:, 1 : J + 1, :].astype(np.float16)

    # 0/1 selection matrix gathering stripe rows {32g+k} -> row 2g+k
    sel = np.zeros((P, K * G), dtype=np.float16)
    for g in range(G):
        for k in range(K):
            sel[32 * g + k, K * g + k] = 1.0

    in_maps = []
    for m in range(n_cores):
        if path == "f16":
            xs = xsrc[m * bl : (m + 1) * bl]  # [bl, J, D] f16 view
        else:
            xs = xsrc[m * bl : (m + 1) * bl, 1 : J + 1, :]  # [bl, J, D] view
        # -> [J, D, bl] = [J, C*P, bl]
        xt = np.ascontiguousarray(xs.transpose(1, 2, 0))
        if path == "hilo":
            hi, lo = _split_hilo(xt)  # [J, C*P, bl] each
            # planes per j: [hi_c0, hi_c1, lo_c0, lo_c1] along the P-axis
            xt = np.concatenate(
                [hi.reshape(J, C * P, bl), lo.reshape(J, C * P, bl)], axis=1
            )
        im = {"xt": xt, "wt": wt, "biasr": bias}
        if path == "f16":
            im["sel"] = sel
        in_maps.append(im)
    return in_maps


def _gather(results, n_cores, bl, path):
    # f16: per-core out [K*G, J*bl/g_n] f16, row 2g+k, col (j, bh, grp, b);
    # else [J, g_n, K, bl//g_n] f32, stripe-major.  Batch row n*512 + b
    # with n = bh*(g_n*n_grp) + grp*g_n + g.
    nb, n_bh, n_n, g_n, n_grp = _layout(path, bl)
    out = np.empty((n_cores * bl, J, K), dtype=np.float32)
    for m, r in enumerate(results):
        if path == "f16":
            o = r["out"].reshape(g_n, K, J, n_bh, n_grp, N_MM)
            # -> [bh, grp, g, b512, J, K] -> [bl, J, K]
            o = o.transpose(3, 4, 0, 5, 2, 1).reshape(bl, J, K)
        else:
            o = r["out"].reshape(J, g_n, K, n_bh, n_grp, N_MM)
            o = o.transpose(3, 4, 1, 5, 0, 2).reshape(bl, J, K)
        out[m * bl : (m + 1) * bl] = o
    return out


def _ensure_ntff_hook():
    """The agent image's antenv lacks axon_hooks; shim it so trace=True can
    register the NTFF profiling hook (see trn_agent_boot.trn_boot)."""
    import sys
    import types

    try:
        from antenv.axon_hooks import get_axon_ntff_profile_hook  # noqa: F401

        return
    except ImportError:
        pass
    import antenv

    mod = types.ModuleType("antenv.axon_hooks")
    mod._hook = None

    def set_axon_ntff_profile_hook(h):
        mod._hook = h

    def get_axon_ntff_profile_hook():
        return mod._hook

    mod.set_axon_ntff_profile_hook = set_axon_ntff_profile_hook
    mod.get_axon_ntff_profile_hook = get_axon_ntff_profile_hook
    sys.modules["antenv.axon_hooks"] = mod
    antenv.axon_hooks = mod
    try:
        from trn_agent_boot.trn_boot import _ntff_profile_via_ctypes

        hook = _ntff_profile_via_ctypes("/opt/axon/libaxon_pjrt.so")
        if hook is not None:
            mod._hook = hook
    except Exception:
        pass


def run(x, W, b, path=None, trace=False, n_cores=M_CORES, bl=None):
    from concourse.bass_utils import run_bass_kernel_spmd

    if trace:
        _ensure_ntff_hook()

    path = path or _PATH
    bl = bl or (x.shape[0] // n_cores)
    x = np.asarray(x, dtype=np.float32)
    W = np.asarray(W, dtype=np.float32)
    b = np.asarray(b, dtype=np.float32)
    nc = _get_nc(path, bl)
    in_maps = _prep_core_inputs(x, W, b, path, n_cores, bl)
    res = run_bass_kernel_spmd(
        nc, in_maps, core_ids=list(range(n_cores)), trace=trace
    )
    out = _gather(res.results, n_cores, bl, path)
    return out, res


def kernel(x, W, b):
    out, _ = run(x, W, b)
    return out

